# revision 1
# baseline (speedup 1.0000x reference)
"""BoxFilter 9x9 mean, TRN2 x8 — v5: overlapping input tiles, no neighbor MMs.

Each output block of <=120 rows is produced from ONE 128-row (or smaller,
clamped at image edges) input tile that already contains the +/-4-row halo.
Per psum half just 3 matmuls against one band weight: rhs = e2[j], e2[j+1]
(odd shift folded into the slice), xb[j+8]. DVE tree is only 2 ops
(e1={0,2}, e2={0,2,4,6}). One [rows,1024] 2-bank PSUM tile per block, one
ScalarE normalize+cast copy, one output DMA.
"""

import threading

import numpy as np

NCORES = 8
B, C, H, W = 16, 3, 1024, 1024
IMGS = B * C
IMGS_PER_CORE = IMGS // NCORES
R = 4
OB = 120  # output rows per full block
NFULL = H // OB  # 8 full blocks
LASTO = H - NFULL * OB  # 64
WPAD = W + 16

# per-image block table: (out_start, out_rows, in_start, in_rows, w_idx, rs_idx)
BLOCKS = []
BLOCKS.append((0, OB, 0, 124, 0, 0))
for I in range(1, NFULL):
    BLOCKS.append((OB * I, OB, OB * I - R, 128, 1, 2))
BLOCKS.append((H - LASTO, LASTO, H - 96, 96, 2, 1))


def _window_counts():
    r = np.arange(H)
    return (np.minimum(r + R, H - 1) - np.maximum(r - R, 0) + 1).astype(np.float32)


def _consts():
    ch = _window_counts()
    k = np.arange(128)[:, None]
    m = np.arange(128)[None, :]
    # W0: tile rows = image rows 0..127; out m needs rows max(0,m-4)..m+4
    w0 = ((np.maximum(m - R, 0) <= k) & (k <= m + R) & (m < OB)).astype(np.float32)
    # W_int: tile rows = image rows s-4..s+123; out m needs tile k = m..m+8
    wi = ((m <= k) & (k <= m + 2 * R) & (m < OB)).astype(np.float32)
    # W8: tile rows = image rows 928..1023 (96); out m (0..63, global 960+m)
    # needs k = 28+m .. min(36+m, 95)
    w8 = ((m + 32 - R <= k) & (k <= np.minimum(m + 32 + R, 95)) & (m < LASTO)).astype(
        np.float32
    )
    wts = np.stack([w0, wi, w8]).astype(np.float16)

    rowscale = np.empty((128, 3), np.float32)
    rowscale[:, 0] = 1.0 / (ch[0:128] * 9.0)          # block 0 (rows 0..119 used)
    rowscale[:, 1] = 1.0 / 81.0
    rowscale[0:LASTO, 1] = 1.0 / (ch[H - LASTO : H] * 9.0)  # block 8 rows at p 0..63
    rowscale[:, 2] = 1.0 / 81.0
    return wts, rowscale


def _build(reps: int = 1):
    import concourse.bacc as bacc
    import concourse.mybir as mybir
    import concourse.tile as tile

    f32 = mybir.dt.float32
    f16 = mybir.dt.float16

    nc = bacc.Bacc("TRN2", target_bir_lowering=False, debug=False, num_devices=NCORES)
    x_d = nc.declare_dram_parameter("x", [IMGS_PER_CORE, H, W], f32, isOutput=False)
    wts_d = nc.declare_dram_parameter("wts", [3, 128, 128], f16, isOutput=False)
    rs_d = nc.declare_dram_parameter("rowscale", [128, 3], f32, isOutput=False)
    o_d = nc.declare_dram_parameter("out", [IMGS_PER_CORE, H, W], f16, isOutput=True)

    with tile.TileContext(nc) as tc:
        with (
            tc.tile_pool(name="consts", bufs=1) as cpool,
            tc.tile_pool(name="xb", bufs=6) as xb_pool,
            tc.tile_pool(name="e1", bufs=4) as e1_pool,
            tc.tile_pool(name="e2", bufs=6) as e2_pool,
            tc.tile_pool(name="osb", bufs=6) as out_pool,
            tc.tile_pool(name="ps", bufs=8, space="PSUM") as ps_pool,
        ):
            w_sb = cpool.tile([128, 3 * 128], f16)
            for i in range(3):
                nc.sync.dma_start(out=w_sb[:, 128 * i : 128 * (i + 1)], in_=wts_d[i])
            rs_sb = cpool.tile([128, 3], f32)
            nc.sync.dma_start(out=rs_sb[:], in_=rs_d[:])

            def block(g, blk):
                os_, orows, is_, irows, wi, si = blk
                xb = xb_pool.tile([128, WPAD], f16, tag="xb")
                nc.gpsimd.memset(xb[0:irows, 0:4], 0.0)
                nc.gpsimd.memset(xb[0:irows, W + 4 : WPAD], 0.0)
                nc.gpsimd.dma_start(
                    out=xb[0:irows, 4 : W + 4], in_=x_d[g, is_ : is_ + irows, :]
                )
                e1 = e1_pool.tile([128, WPAD - 2], f16, tag="e1")
                nc.vector.tensor_add(
                    out=e1[0:irows, :],
                    in0=xb[0:irows, 0 : WPAD - 2],
                    in1=xb[0:irows, 2:WPAD],
                )
                e2 = e2_pool.tile([128, WPAD - 6], f16, tag="e2")
                nc.vector.tensor_add(
                    out=e2[0:irows, :],
                    in0=e1[0:irows, 0 : WPAD - 6],
                    in1=e1[0:irows, 4 : WPAD - 2],
                )

                out_sb = out_pool.tile([128, W], f16, tag="osb")
                wv = w_sb[0:irows, 128 * wi : 128 * wi + orows]
                rsv = rs_sb[0:orows, si : si + 1]
                for h in range(2):
                    j0 = 512 * h
                    ps = ps_pool.tile([128, 512], f32, tag="ps", name=f"ps{h}")
                    nc.tensor.matmul(
                        ps[0:orows, :],
                        wv,
                        e2[0:irows, j0 : j0 + 512],
                        start=True,
                        stop=False,
                    )
                    nc.tensor.matmul(
                        ps[0:orows, :],
                        wv,
                        e2[0:irows, j0 + 1 : j0 + 513],
                        start=False,
                        stop=False,
                    )
                    nc.tensor.matmul(
                        ps[0:orows, :],
                        wv,
                        xb[0:irows, j0 + 8 : j0 + 520],
                        start=False,
                        stop=True,
                    )
                    nc.scalar.mul(
                        out_sb[0:orows, j0 : j0 + 512], ps[0:orows, :], rsv
                    )
                nc.scalar.dma_start(
                    out=o_d[g, os_ : os_ + orows, :], in_=out_sb[0:orows, :]
                )

            for _ in range(reps):
                for g in range(IMGS_PER_CORE):
                    for blk in BLOCKS:
                        block(g, blk)

    nc.compile()
    return nc


_LOCK = threading.Lock()
_CACHED = {}


def _get_nc(reps: int = 1):
    with _LOCK:
        key = ("nc", reps)
        if key not in _CACHED:
            _CACHED[key] = _build(reps)
        return _CACHED[key]


def _postprocess(out48_f16: np.ndarray) -> np.ndarray:
    out = out48_f16.astype(np.float32).reshape(B, C, H, W)
    ch = _window_counts()
    out[..., 0:R] *= (9.0 / ch[0:R])[None, None, None, :]
    out[..., W - R : W] *= (9.0 / ch[H - R : H])[None, None, None, :]
    return out


def run(x: np.ndarray, trace: bool = False, reps: int = 1):
    from concourse.bass_utils import run_bass_kernel_spmd

    assert x.shape == (B, C, H, W), x.shape
    x48 = np.ascontiguousarray(x.reshape(IMGS, H, W), dtype=np.float32)
    wts, rowscale = _consts()
    in_maps = [
        {
            "x": np.ascontiguousarray(
                x48[IMGS_PER_CORE * c : IMGS_PER_CORE * (c + 1)]
            ),
            "wts": wts,
            "rowscale": rowscale,
        }
        for c in range(NCORES)
    ]
    nc = _get_nc(reps)
    res = run_bass_kernel_spmd(
        nc, in_maps, core_ids=list(range(NCORES)), trace=trace
    )
    out48 = np.concatenate([r["out"] for r in res.results], axis=0)
    return _postprocess(out48), res


def kernel(x: np.ndarray) -> np.ndarray:
    out, _ = run(x, trace=False)
    return out



# revision 4
# speedup vs baseline: 1.0419x; 1.0419x over previous
"""BoxFilter 9x9 mean, TRN2 x8 — v6: scan + matmul-differencing.

Per 128-row block:
  - casting DMA loads x f32->fp8 (or f16) into a persistent 9-chunk SBUF tile
    (chunks padded with 5 left + 4 right zero cols, zeroed once at startup)
  - one DVE tensor_tensor_scan produces the horizontal prefix sum c (f16,
    fp32 internal state): h[j] = c[j+9] - c[j] is the 9-wide window sum
  - 4 matmuls per block compute the vertical 9-band sum of h directly from c:
    ps = (+W)@c[:, 9:] + (-W)@c[:, :1024], with the 1/(9*vcount) row scale
    folded into W
  - one 1024-wide scalar-engine copy drains the 2-bank PSUM tile to SBUF f16
  - batched DMAs write the per-image output back (f16), edge-column scale
    fixed on host

Engine budget per core (TimelineSim): DVE ~61us (scans), Act ~56us (drains),
DMA ~54us (fp8 in + f16 out), PE.SEQ ~54us (4 MM/block), Pool ~20us (SWDGE).
"""

import threading

import numpy as np

NCORES = 8
B, C, H, W = 16, 3, 1024, 1024
IMGS = B * C
IPC = IMGS // NCORES
R = 4
OB = 120
NBLK = 9
P = 1040  # SBUF chunk pitch (elements)
CW = 1033  # scan width: 5 left zeros + 1024 + 4 right zeros

# (out_start, out_rows, in_start, in_rows, weight_kind)
BLOCKS = [(0, OB, 0, 124, 0)]
BLOCKS += [(OB * t, OB, OB * t - R, 128, 1) for t in range(1, 8)]
BLOCKS += [(960, 64, 956, 68, 2)]

INPUT_FP8 = False  # False -> f16 input path
POOL_SCANS = 0  # chunks per image scanned on gpsimd instead of DVE
DVE_DRAINS = 0  # drains per image on DVE instead of Act


def _weights():
    k = np.arange(128)[:, None].astype(np.int64)
    m = np.arange(128)[None, :].astype(np.int64)
    vc0 = np.minimum(m + R, 1023) - np.maximum(m - R, 0) + 1
    w0 = ((np.maximum(m - R, 0) <= k) & (k <= m + R) & (m < OB)) / (9.0 * vc0)
    wi = ((m <= k) & (k <= m + 2 * R) & (m < OB)) / 81.0
    vc8 = np.maximum(np.minimum(964 + m, 1023) - (956 + m) + 1, 1)
    w8 = ((m <= k) & (k <= np.minimum(m + 2 * R, 67)) & (m < 64)) / (9.0 * vc8)
    wts = np.stack([w0, wi, w8])
    return np.concatenate([wts, -wts]).astype(np.float16)  # [6,128,128]


def _build(reps: int = 1):
    import concourse.bacc as bacc
    import concourse.mybir as mybir
    import concourse.tile as tile
    from concourse.ap import AP

    f32 = mybir.dt.float32
    f16 = mybir.dt.float16
    fp8 = mybir.dt.float8e4
    xdt = fp8 if INPUT_FP8 else f16
    add = mybir.AluOpType.add

    nc = bacc.Bacc("TRN2", target_bir_lowering=False, debug=False, num_devices=NCORES)
    x_d = nc.declare_dram_parameter("x", [IPC, H, W], f32, isOutput=False)
    wts_d = nc.declare_dram_parameter("wts", [6, 128, 128], f16, isOutput=False)
    o_d = nc.declare_dram_parameter("out", [IPC, H, W], f16, isOutput=True)

    with tile.TileContext(nc) as tc:
        with (
            tc.tile_pool(name="consts", bufs=1) as cpool,
            tc.tile_pool(name="xb", bufs=2) as xb_pool,
            tc.tile_pool(name="cs", bufs=4) as c_pool,
            tc.tile_pool(name="ob", bufs=2) as ob_pool,
            tc.tile_pool(name="ps", bufs=4, space="PSUM") as ps_pool,
        ):
            w_sb = cpool.tile([128, 6 * 128], f16)
            for i in range(6):
                nc.sync.dma_start(out=w_sb[:, 128 * i : 128 * (i + 1)], in_=wts_d[i])
            zeros = cpool.tile([128, CW], f16)
            nc.gpsimd.memset(zeros[:, :], 0.0)

            # persistent double-buffered input / output tiles
            xbs = [xb_pool.tile([128, NBLK * P], xdt, tag="xb", name=f"xb{i}")
                   for i in range(2)]
            obs = [ob_pool.tile([128, NBLK * 1024], f16, tag="ob", name=f"ob{i}")
                   for i in range(2)]
            for xb in xbs:
                nat = xb[0:128, 0 : NBLK * P]
                pstride = nat.ap[0][0]
                padl = AP(nat.tensor, nat.offset, [[pstride, 128], [P, NBLK], [1, 5]])
                padr = AP(nat.tensor, nat.offset + 1029,
                          [[pstride, 128], [P, NBLK], [1, 11]])
                nc.gpsimd.memset(padl, 0.0)
                nc.gpsimd.memset(padr, 0.0)

            def image(g):
                xb = xbs[g % 2]
                ob = obs[g % 2]
                nat = xb[0:128, 0 : NBLK * P]
                pstride = nat.ap[0][0]
                # input DMAs: t0, batched t1..7, t8  (casting f32 -> xdt)
                nc.gpsimd.dma_start(out=xb[0:124, 5:1029], in_=x_d[g, 0:124, :])
                dimg = x_d[g]
                dsrc = AP(dimg.tensor, dimg.offset + 116 * W,
                          [[W, 128], [OB * W, 7], [1, W]])
                ddst = AP(nat.tensor, nat.offset + P + 5,
                          [[pstride, 128], [P, 7], [1, W]])
                nc.gpsimd.dma_start(out=ddst, in_=dsrc)
                nc.gpsimd.dma_start(
                    out=xb[0:68, 8 * P + 5 : 8 * P + 1029], in_=x_d[g, 956:1024, :]
                )

                for t, (os_, orows, is_, irows, wk) in enumerate(BLOCKS):
                    c = c_pool.tile([128, CW], f16, tag="c")
                    eng = nc.gpsimd if t < POOL_SCANS else nc.vector
                    eng.tensor_tensor_scan(
                        out=c[0:irows, :],
                        data0=zeros[0:irows, :],
                        data1=xb[0:irows, P * t : P * t + CW],
                        initial=0.0,
                        op0=add,
                        op1=add,
                    )
                    ps = ps_pool.tile([128, 1024], f32, tag="ps")
                    wp = w_sb[0:irows, 128 * wk : 128 * wk + orows]
                    wm = w_sb[0:irows, 128 * (wk + 3) : 128 * (wk + 3) + orows]
                    nc.tensor.matmul(ps[0:orows, 0:512], wp, c[0:irows, 9:521],
                                     start=True, stop=False)
                    nc.tensor.matmul(ps[0:orows, 0:512], wm, c[0:irows, 0:512],
                                     start=False, stop=True)
                    nc.tensor.matmul(ps[0:orows, 512:1024], wp, c[0:irows, 521:1033],
                                     start=True, stop=False)
                    nc.tensor.matmul(ps[0:orows, 512:1024], wm, c[0:irows, 512:1024],
                                     start=False, stop=True)
                    dst = ob[0:orows, 1024 * t : 1024 * t + 1024]
                    if t < DVE_DRAINS:
                        nc.vector.tensor_copy(dst, ps[0:orows, :])
                    else:
                        nc.scalar.copy(dst, ps[0:orows, :])

                # output DMAs: batched t0..7, then t8
                dout = o_d[g]
                ddram = AP(dout.tensor, dout.offset,
                           [[W, OB], [OB * W, 8], [1, W]])
                onat = ob[0:128, 0 : NBLK * 1024]
                opstride = onat.ap[0][0]
                osrc = AP(onat.tensor, onat.offset,
                          [[opstride, OB], [1024, 8], [1, 1024]])
                nc.sync.dma_start(out=ddram, in_=osrc)
                nc.sync.dma_start(
                    out=o_d[g, 960:1024, :], in_=ob[0:64, 8 * 1024 : 9 * 1024]
                )

            for _ in range(reps):
                for g in range(IPC):
                    image(g)

    nc.compile()
    return nc


_LOCK = threading.Lock()
_CACHED = {}


def _get_nc(reps: int = 1):
    with _LOCK:
        key = ("nc", reps)
        if key not in _CACHED:
            _CACHED[key] = _build(reps)
        return _CACHED[key]


def _postprocess(out48_f16: np.ndarray) -> np.ndarray:
    out = out48_f16.astype(np.float32).reshape(B, C, H, W)
    r = np.arange(H)
    hc = (np.minimum(r + R, W - 1) - np.maximum(r - R, 0) + 1).astype(np.float32)
    out[..., 0:R] *= (9.0 / hc[0:R])[None, None, None, :]
    out[..., W - R : W] *= (9.0 / hc[W - R : W])[None, None, None, :]
    return out


def run(x: np.ndarray, trace: bool = False, reps: int = 1):
    from concourse.bass_utils import run_bass_kernel_spmd

    assert x.shape == (B, C, H, W), x.shape
    x48 = np.ascontiguousarray(x.reshape(IMGS, H, W), dtype=np.float32)
    wts = _weights()
    in_maps = [
        {
            "x": np.ascontiguousarray(x48[IPC * c : IPC * (c + 1)]),
            "wts": wts,
        }
        for c in range(NCORES)
    ]
    nc = _get_nc(reps)
    res = run_bass_kernel_spmd(nc, in_maps, core_ids=list(range(NCORES)), trace=trace)
    out48 = np.concatenate([r["out"] for r in res.results], axis=0)
    return _postprocess(out48), res


def kernel(x: np.ndarray) -> np.ndarray:
    out, _ = run(x, trace=False)
    return out


# revision 8
# speedup vs baseline: 1.1795x; 1.1321x over previous
"""BoxFilter 9x9 mean, TRN2 x8 — v6: scan + matmul-differencing.

Per 128-row block:
  - casting DMA loads x f32->fp8 (or f16) into a persistent 9-chunk SBUF tile
    (chunks padded with 5 left + 4 right zero cols, zeroed once at startup)
  - one DVE tensor_tensor_scan produces the horizontal prefix sum c (f16,
    fp32 internal state): h[j] = c[j+9] - c[j] is the 9-wide window sum
  - 4 matmuls per block compute the vertical 9-band sum of h directly from c:
    ps = (+W)@c[:, 9:] + (-W)@c[:, :1024], with the 1/(9*vcount) row scale
    folded into W
  - one 1024-wide scalar-engine copy drains the 2-bank PSUM tile to SBUF f16
  - batched DMAs write the per-image output back (f16), edge-column scale
    fixed on host

Engine budget per core (TimelineSim): DVE ~61us (scans), Act ~56us (drains),
DMA ~54us (fp8 in + f16 out), PE.SEQ ~54us (4 MM/block), Pool ~20us (SWDGE).
"""

import threading

import numpy as np

NCORES = 8
B, C, H, W = 16, 3, 1024, 1024
IMGS = B * C
IPC = IMGS // NCORES
R = 4
OB = 120
NBLK = 9
P = 1040  # SBUF chunk pitch (elements)
CW = 1033  # scan width: 5 left zeros + 1024 + 4 right zeros

# (out_start, out_rows, in_start, in_rows, weight_kind)
BLOCKS = [(0, OB, 0, 124, 0)]
BLOCKS += [(OB * t, OB, OB * t - R, 128, 1) for t in range(1, 8)]
BLOCKS += [(960, 64, 956, 68, 2)]

INPUT_FP8 = False  # False -> f16 input path
POOL_SCANS = 0  # chunks per image scanned on gpsimd instead of DVE
DVE_DRAINS = 0  # drains per image on DVE instead of Act


def _weights():
    k = np.arange(128)[:, None].astype(np.int64)
    m = np.arange(128)[None, :].astype(np.int64)
    vc0 = np.minimum(m + R, 1023) - np.maximum(m - R, 0) + 1
    w0 = ((np.maximum(m - R, 0) <= k) & (k <= m + R) & (m < OB)) / (9.0 * vc0)
    wi = ((m <= k) & (k <= m + 2 * R) & (m < OB)) / 81.0
    vc8 = np.maximum(np.minimum(964 + m, 1023) - (956 + m) + 1, 1)
    w8 = ((m <= k) & (k <= np.minimum(m + 2 * R, 67)) & (m < 64)) / (9.0 * vc8)
    wts = np.stack([w0, wi, w8])
    wts6 = np.concatenate([wts, -wts]).astype(np.float16)  # [6,128,128]
    return np.ascontiguousarray(wts6.transpose(1, 0, 2).reshape(128, 6 * 128))


def _build(reps: int = 1):
    import concourse.bacc as bacc
    import concourse.mybir as mybir
    import concourse.tile as tile
    from concourse.ap import AP

    f32 = mybir.dt.float32
    f16 = mybir.dt.float16
    fp8 = mybir.dt.float8e4
    xdt = fp8 if INPUT_FP8 else f16
    add = mybir.AluOpType.add

    nc = bacc.Bacc("TRN2", target_bir_lowering=False, debug=False, num_devices=NCORES)
    x_d = nc.declare_dram_parameter("x", [IPC, H, W], f32, isOutput=False)
    wts_d = nc.declare_dram_parameter("wts", [128, 6 * 128], f16, isOutput=False)
    o_d = nc.declare_dram_parameter("out", [IPC, H, W], f16, isOutput=True)

    NXB = 3
    NOB = 3
    with tile.TileContext(nc) as tc:
        with (
            tc.tile_pool(name="consts", bufs=1) as cpool,
            tc.tile_pool(name="xb", bufs=NXB) as xb_pool,
            tc.tile_pool(name="cs", bufs=4) as c_pool,
            tc.tile_pool(name="ob", bufs=NOB) as ob_pool,
            tc.tile_pool(name="ps", bufs=4, space="PSUM") as ps_pool,
        ):
            w_sb = cpool.tile([128, 6 * 128], f16)
            nc.sync.dma_start(out=w_sb[:, :], in_=wts_d[:, :])
            zeros = cpool.tile([128, CW], f16)
            nc.gpsimd.memset(zeros[:, :], 0.0)

            # persistent multi-buffered input / output tiles
            xbs = [xb_pool.tile([128, NBLK * P], xdt, tag="xb", name=f"xb{i}")
                   for i in range(NXB)]
            obs = [ob_pool.tile([128, NBLK * 1024], f16, tag="ob", name=f"ob{i}")
                   for i in range(NOB)]
            for xb in xbs:
                nat = xb[0:128, 0 : NBLK * P]
                pstride = nat.ap[0][0]
                padl = AP(nat.tensor, nat.offset, [[pstride, 128], [P, NBLK], [1, 5]])
                padr = AP(nat.tensor, nat.offset + 1029,
                          [[pstride, 128], [P, NBLK], [1, 11]])
                nc.gpsimd.memset(padl, 0.0)
                nc.gpsimd.memset(padr, 0.0)

            def load_image(g):
                xb = xbs[g % NXB]
                nat = xb[0:128, 0 : NBLK * P]
                pstride = nat.ap[0][0]
                # input DMAs: t0, batched t1..4, t5..7, t8 (casting f32 -> xdt)
                nc.gpsimd.dma_start(out=xb[0:124, 5:1029], in_=x_d[g, 0:124, :])
                dimg = x_d[g]
                for lo, n in ((1, 4), (5, 3)):
                    dsrc = AP(dimg.tensor, dimg.offset + (OB * lo - R) * W,
                              [[W, 128], [OB * W, n], [1, W]])
                    ddst = AP(nat.tensor, nat.offset + lo * P + 5,
                              [[pstride, 128], [P, n], [1, W]])
                    nc.gpsimd.dma_start(out=ddst, in_=dsrc)
                nc.gpsimd.dma_start(
                    out=xb[0:68, 8 * P + 5 : 8 * P + 1029], in_=x_d[g, 956:1024, :]
                )

            def image(g):
                xb = xbs[g % NXB]
                ob = obs[g % NOB]

                for t, (os_, orows, is_, irows, wk) in enumerate(BLOCKS):
                    c = c_pool.tile([128, CW], f16, tag="c")
                    eng = nc.gpsimd if t < POOL_SCANS else nc.vector
                    eng.tensor_tensor_scan(
                        out=c[0:irows, :],
                        data0=zeros[0:irows, :],
                        data1=xb[0:irows, P * t : P * t + CW],
                        initial=0.0,
                        op0=add,
                        op1=add,
                    )
                    ps = ps_pool.tile([128, 1024], f32, tag="ps")
                    wp = w_sb[0:irows, 128 * wk : 128 * wk + orows]
                    wm = w_sb[0:irows, 128 * (wk + 3) : 128 * (wk + 3) + orows]
                    nc.tensor.matmul(ps[0:orows, 0:512], wp, c[0:irows, 9:521],
                                     start=True, stop=False)
                    nc.tensor.matmul(ps[0:orows, 0:512], wm, c[0:irows, 0:512],
                                     start=False, stop=True)
                    nc.tensor.matmul(ps[0:orows, 512:1024], wp, c[0:irows, 521:1033],
                                     start=True, stop=False)
                    nc.tensor.matmul(ps[0:orows, 512:1024], wm, c[0:irows, 512:1024],
                                     start=False, stop=True)
                    dst = ob[0:orows, 1024 * t : 1024 * t + 1024]
                    if t < DVE_DRAINS:
                        nc.vector.tensor_copy(dst, ps[0:orows, :])
                    else:
                        nc.scalar.copy(dst, ps[0:orows, :])

                # output DMAs: batched t0..3, t4..7, then t8
                dout = o_d[g]
                onat = ob[0:128, 0 : NBLK * 1024]
                opstride = onat.ap[0][0]
                for lo, n in ((0, 4), (4, 4)):
                    ddram = AP(dout.tensor, dout.offset + OB * lo * W,
                               [[W, OB], [OB * W, n], [1, W]])
                    osrc = AP(onat.tensor, onat.offset + lo * 1024,
                              [[opstride, OB], [1024, n], [1, 1024]])
                    nc.sync.dma_start(out=ddram, in_=osrc)
                nc.sync.dma_start(
                    out=o_d[g, 960:1024, :], in_=ob[0:64, 8 * 1024 : 9 * 1024]
                )

            for _ in range(reps):
                load_image(0)
                load_image(1)
                for g in range(IPC):
                    if g + 2 < IPC:
                        load_image(g + 2)
                    image(g)

    nc.compile()
    return nc


_LOCK = threading.Lock()
_CACHED = {}


def _get_nc(reps: int = 1):
    with _LOCK:
        key = ("nc", reps)
        if key not in _CACHED:
            _CACHED[key] = _build(reps)
        return _CACHED[key]


def _postprocess(out48_f16: np.ndarray) -> np.ndarray:
    out = out48_f16.astype(np.float32).reshape(B, C, H, W)
    r = np.arange(H)
    hc = (np.minimum(r + R, W - 1) - np.maximum(r - R, 0) + 1).astype(np.float32)
    out[..., 0:R] *= (9.0 / hc[0:R])[None, None, None, :]
    out[..., W - R : W] *= (9.0 / hc[W - R : W])[None, None, None, :]
    return out


def run(x: np.ndarray, trace: bool = False, reps: int = 1):
    from concourse.bass_utils import run_bass_kernel_spmd

    assert x.shape == (B, C, H, W), x.shape
    x48 = np.ascontiguousarray(x.reshape(IMGS, H, W), dtype=np.float32)
    wts = _weights()
    in_maps = [
        {
            "x": np.ascontiguousarray(x48[IPC * c : IPC * (c + 1)]),
            "wts": wts,
        }
        for c in range(NCORES)
    ]
    nc = _get_nc(reps)
    res = run_bass_kernel_spmd(nc, in_maps, core_ids=list(range(NCORES)), trace=trace)
    out48 = np.concatenate([r["out"] for r in res.results], axis=0)
    return _postprocess(out48), res


def kernel(x: np.ndarray) -> np.ndarray:
    out, _ = run(x, trace=False)
    return out


# revision 11
# speedup vs baseline: 1.2120x; 1.0275x over previous
"""BoxFilter 9x9 mean, TRN2 x8 — v6: scan + matmul-differencing.

Per 128-row block:
  - casting DMA loads x f32->fp8 (or f16) into a persistent 9-chunk SBUF tile
    (chunks padded with 5 left + 4 right zero cols, zeroed once at startup)
  - one DVE tensor_tensor_scan produces the horizontal prefix sum c (f16,
    fp32 internal state): h[j] = c[j+9] - c[j] is the 9-wide window sum
  - 4 matmuls per block compute the vertical 9-band sum of h directly from c:
    ps = (+W)@c[:, 9:] + (-W)@c[:, :1024], with the 1/(9*vcount) row scale
    folded into W
  - one 1024-wide scalar-engine copy drains the 2-bank PSUM tile to SBUF f16
  - batched DMAs write the per-image output back (f16), edge-column scale
    fixed on host

Engine budget per core (TimelineSim): DVE ~61us (scans), Act ~56us (drains),
DMA ~54us (fp8 in + f16 out), PE.SEQ ~54us (4 MM/block), Pool ~20us (SWDGE).
"""

import threading

import numpy as np

NCORES = 8
B, C, H, W = 16, 3, 1024, 1024
IMGS = B * C
IPC = IMGS // NCORES
R = 4
OB = 120
NBLK = 9
P = 1040  # SBUF chunk pitch (elements)
CW = 1033  # scan width: 5 left zeros + 1024 + 4 right zeros

# (out_start, out_rows, in_start, in_rows, weight_kind)
BLOCKS = [(0, OB, 0, 124, 0)]
BLOCKS += [(OB * t, OB, OB * t - R, 128, 1) for t in range(1, 8)]
BLOCKS += [(960, 64, 956, 68, 2)]

INPUT_FP8 = False  # False -> f16 input path
POOL_SCANS = 0  # chunks per image scanned on gpsimd instead of DVE
DVE_DRAINS = 0  # drains per image on DVE instead of Act


def _weights():
    k = np.arange(128)[:, None].astype(np.int64)
    m = np.arange(128)[None, :].astype(np.int64)
    vc0 = np.minimum(m + R, 1023) - np.maximum(m - R, 0) + 1
    w0 = ((np.maximum(m - R, 0) <= k) & (k <= m + R) & (m < OB)) / (9.0 * vc0)
    wi = ((m <= k) & (k <= m + 2 * R) & (m < OB)) / 81.0
    vc8 = np.maximum(np.minimum(964 + m, 1023) - (956 + m) + 1, 1)
    w8 = ((m <= k) & (k <= np.minimum(m + 2 * R, 67)) & (m < 64)) / (9.0 * vc8)
    wts = np.stack([w0, wi, w8])
    wts6 = np.concatenate([wts, -wts]).astype(np.float16)  # [6,128,128]
    return np.ascontiguousarray(wts6.transpose(1, 0, 2).reshape(128, 6 * 128))


def _build(reps: int = 1):
    import concourse.bacc as bacc
    import concourse.mybir as mybir
    import concourse.tile as tile
    from concourse.ap import AP

    f32 = mybir.dt.float32
    f16 = mybir.dt.float16
    fp8 = mybir.dt.float8e4
    xdt = fp8 if INPUT_FP8 else f16
    add = mybir.AluOpType.add

    nc = bacc.Bacc("TRN2", target_bir_lowering=False, debug=False, num_devices=NCORES)
    x_d = nc.declare_dram_parameter("x", [IPC, H, W], f32, isOutput=False)
    wts_d = nc.declare_dram_parameter("wts", [128, 6 * 128], f16, isOutput=False)
    o_d = nc.declare_dram_parameter("out", [IPC, H, W], f16, isOutput=True)

    NXB = 3
    NOB = 3
    with tile.TileContext(nc) as tc:
        with (
            tc.tile_pool(name="consts", bufs=1) as cpool,
            tc.tile_pool(name="xb", bufs=NXB) as xb_pool,
            tc.tile_pool(name="cs", bufs=4) as c_pool,
            tc.tile_pool(name="ob", bufs=NOB) as ob_pool,
            tc.tile_pool(name="ps", bufs=4, space="PSUM") as ps_pool,
        ):
            w_sb = cpool.tile([128, 6 * 128], f16)
            nc.sync.dma_start(out=w_sb[:, :], in_=wts_d[:, :])
            zeros = cpool.tile([128, CW], f16)
            nc.vector.memset(zeros[:, :], 0.0)

            # persistent multi-buffered input / output tiles
            xbs = [xb_pool.tile([128, NBLK * P], xdt, tag="xb", name=f"xb{i}")
                   for i in range(NXB)]
            obs = [ob_pool.tile([128, NBLK * 1024], f16, tag="ob", name=f"ob{i}")
                   for i in range(NOB)]
            for xb in xbs:
                nat = xb[0:128, 0 : NBLK * P]
                pstride = nat.ap[0][0]
                padl = AP(nat.tensor, nat.offset, [[pstride, 128], [P, NBLK], [1, 5]])
                padr = AP(nat.tensor, nat.offset + 1029,
                          [[pstride, 128], [P, NBLK], [1, 11]])
                nc.vector.memset(padl, 0.0)
                nc.vector.memset(padr, 0.0)

            def load_image(g):
                xb = xbs[g % NXB]
                nat = xb[0:128, 0 : NBLK * P]
                pstride = nat.ap[0][0]
                # input DMAs: t0, batched t1..4, t5..7, t8 (casting f32 -> xdt)
                nc.gpsimd.dma_start(out=xb[0:124, 5:1029], in_=x_d[g, 0:124, :])
                dimg = x_d[g]
                for lo, n in ((1, 4), (5, 3)):
                    dsrc = AP(dimg.tensor, dimg.offset + (OB * lo - R) * W,
                              [[W, 128], [OB * W, n], [1, W]])
                    ddst = AP(nat.tensor, nat.offset + lo * P + 5,
                              [[pstride, 128], [P, n], [1, W]])
                    nc.gpsimd.dma_start(out=ddst, in_=dsrc)
                nc.gpsimd.dma_start(
                    out=xb[0:68, 8 * P + 5 : 8 * P + 1029], in_=x_d[g, 956:1024, :]
                )

            def image(g):
                xb = xbs[g % NXB]
                ob = obs[g % NOB]

                for t, (os_, orows, is_, irows, wk) in enumerate(BLOCKS):
                    c = c_pool.tile([128, CW], f16, tag="c")
                    eng = nc.gpsimd if t < POOL_SCANS else nc.vector
                    eng.tensor_tensor_scan(
                        out=c[0:irows, :],
                        data0=zeros[0:irows, :],
                        data1=xb[0:irows, P * t : P * t + CW],
                        initial=0.0,
                        op0=add,
                        op1=add,
                    )
                    ps = ps_pool.tile([128, 1024], f32, tag="ps")
                    wp = w_sb[0:irows, 128 * wk : 128 * wk + orows]
                    wm = w_sb[0:irows, 128 * (wk + 3) : 128 * (wk + 3) + orows]
                    nc.tensor.matmul(ps[0:orows, 0:512], wp, c[0:irows, 9:521],
                                     start=True, stop=False)
                    nc.tensor.matmul(ps[0:orows, 0:512], wm, c[0:irows, 0:512],
                                     start=False, stop=True)
                    nc.tensor.matmul(ps[0:orows, 512:1024], wp, c[0:irows, 521:1033],
                                     start=True, stop=False)
                    nc.tensor.matmul(ps[0:orows, 512:1024], wm, c[0:irows, 512:1024],
                                     start=False, stop=True)
                    dst = ob[0:orows, 1024 * t : 1024 * t + 1024]
                    if t < DVE_DRAINS:
                        nc.vector.tensor_copy(dst, ps[0:orows, :])
                    else:
                        nc.scalar.copy(dst, ps[0:orows, :])

                # output DMAs: batched chunk pairs, then t8
                dout = o_d[g]
                onat = ob[0:128, 0 : NBLK * 1024]
                opstride = onat.ap[0][0]
                for lo, n in ((0, 2), (2, 2), (4, 2), (6, 2)):
                    ddram = AP(dout.tensor, dout.offset + OB * lo * W,
                               [[W, OB], [OB * W, n], [1, W]])
                    osrc = AP(onat.tensor, onat.offset + lo * 1024,
                              [[opstride, OB], [1024, n], [1, 1024]])
                    nc.sync.dma_start(out=ddram, in_=osrc)
                nc.sync.dma_start(
                    out=o_d[g, 960:1024, :], in_=ob[0:64, 8 * 1024 : 9 * 1024]
                )

            for _ in range(reps):
                load_image(0)
                load_image(1)
                for g in range(IPC):
                    if g + 2 < IPC:
                        load_image(g + 2)
                    image(g)

    nc.compile()
    return nc


_LOCK = threading.Lock()
_CACHED = {}


def _get_nc(reps: int = 1):
    with _LOCK:
        key = ("nc", reps)
        if key not in _CACHED:
            _CACHED[key] = _build(reps)
        return _CACHED[key]


def _postprocess(out48_f16: np.ndarray) -> np.ndarray:
    out = out48_f16.astype(np.float32).reshape(B, C, H, W)
    r = np.arange(H)
    hc = (np.minimum(r + R, W - 1) - np.maximum(r - R, 0) + 1).astype(np.float32)
    out[..., 0:R] *= (9.0 / hc[0:R])[None, None, None, :]
    out[..., W - R : W] *= (9.0 / hc[W - R : W])[None, None, None, :]
    return out


def run(x: np.ndarray, trace: bool = False, reps: int = 1):
    from concourse.bass_utils import run_bass_kernel_spmd

    assert x.shape == (B, C, H, W), x.shape
    x48 = np.ascontiguousarray(x.reshape(IMGS, H, W), dtype=np.float32)
    wts = _weights()
    in_maps = [
        {
            "x": np.ascontiguousarray(x48[IPC * c : IPC * (c + 1)]),
            "wts": wts,
        }
        for c in range(NCORES)
    ]
    nc = _get_nc(reps)
    res = run_bass_kernel_spmd(nc, in_maps, core_ids=list(range(NCORES)), trace=trace)
    out48 = np.concatenate([r["out"] for r in res.results], axis=0)
    return _postprocess(out48), res


def kernel(x: np.ndarray) -> np.ndarray:
    out, _ = run(x, trace=False)
    return out
